# revision 1
# baseline (speedup 1.0000x reference)
"""Top-1 nearest-neighbor retrieval kernel for Trainium2 (8 NeuronCores).

Reference computation:
    dis = sum((db_vel - in_vel)**2, axis=1)   # [N]
    ind = argmin(dis)
    out = pred_vel[ind][None, :]

Strategy (memory-bound):
  - Shard db_vel row-wise: 100000 rows -> 8 cores x 12500 rows.
  - Each core streams its 12500x1056 f32 shard from HBM in large DMAs and
    computes per-row squared distance to the query:
        DVE:     diff = db_tile - q_broadcast          (tensor_tensor sub)
        ScalarE: sq = diff**2, dis_col = sum(sq)       (activation Square
                                                        with fused accum_out)
    Both engines do exactly one pass over the data, comfortably under the
    ~358 GB/s/core HBM roofline, so the kernel is DMA-bound.
  - Each core writes a tiny [128, 98] distance tile; the host does the final
    argmin over 100k scalars and gathers the pred_vel row (pred_vel never
    needs to touch the device - only one of its rows is ever read).
"""

import numpy as np

N_DB = 100000
D_IN = 1056
N_CORES = 8
ROWS = N_DB // N_CORES          # 12500 rows per core
P = 128                         # SBUF partitions
NTILES = (ROWS + P - 1) // P    # 98 row-tiles per core (97 full + 84 rows)
CHUNK = 7                       # row-tiles per DMA (3.78 MB per transfer)
NCHUNKS = NTILES // CHUNK       # 14

_CACHE = {}


def _build():
    """Trace + compile the per-core Bass program (cached)."""
    if "nc" in _CACHE:
        return _CACHE["nc"]

    import concourse.bacc as bacc
    import concourse.mybir as mybir
    from concourse.tile import TileContext

    nc = bacc.Bacc(trn_type="TRN2", debug=False)
    db = nc.dram_tensor("db", [ROWS, D_IN], mybir.dt.float32, kind="ExternalInput").ap()
    q = nc.dram_tensor("q", [1, D_IN], mybir.dt.float32, kind="ExternalInput").ap()
    dis = nc.dram_tensor(
        "dis", [P, NTILES], mybir.dt.float32, kind="ExternalOutput"
    ).ap()

    with TileContext(nc) as tc:
        with (
            tc.tile_pool(name="const", bufs=1) as cpool,
            tc.tile_pool(name="chunks", bufs=3) as chpool,
            tc.tile_pool(name="work", bufs=3) as wpool,
            tc.tile_pool(name="acc", bufs=1) as apool,
        ):
            # Broadcast the query across all 128 partitions once.
            qb = cpool.tile([P, D_IN], mybir.dt.float32)
            nc.sync.dma_start(out=qb[:1, :], in_=q[:, :])
            nc.gpsimd.partition_broadcast(qb[:, :], qb[:1, :])

            dis_sb = apool.tile([P, NTILES], mybir.dt.float32)
            nc.vector.memset(dis_sb[:, :], 0.0)

            for c in range(NCHUNKS):
                r0 = c * CHUNK * P
                chunk = chpool.tile([P, CHUNK, D_IN], mybir.dt.float32)
                if c < NCHUNKS - 1:
                    nc.sync.dma_start(
                        out=chunk[:, :, :],
                        in_=db[r0 : r0 + CHUNK * P, :].rearrange(
                            "(a p) d -> p a d", p=P
                        ),
                    )
                    subtiles = [(a, P) for a in range(CHUNK)]
                else:
                    # Tail: 6 full tiles + one 84-row tile (12500 = 97*128+84).
                    full = CHUNK - 1
                    tail = ROWS - (r0 + full * P)
                    nc.sync.dma_start(
                        out=chunk[:, :full, :],
                        in_=db[r0 : r0 + full * P, :].rearrange(
                            "(a p) d -> p a d", p=P
                        ),
                    )
                    nc.sync.dma_start(
                        out=chunk[:tail, full, :],
                        in_=db[r0 + full * P : ROWS, :],
                    )
                    subtiles = [(a, P) for a in range(full)] + [(full, tail)]

                for a, rows in subtiles:
                    t = c * CHUNK + a
                    diff = wpool.tile([P, D_IN], mybir.dt.float32, tag="diff")
                    nc.vector.tensor_sub(
                        out=diff[:rows, :], in0=chunk[:rows, a, :], in1=qb[:rows, :]
                    )
                    sq = wpool.tile([P, D_IN], mybir.dt.float32, tag="sq")
                    nc.scalar.activation(
                        out=sq[:rows, :],
                        in_=diff[:rows, :],
                        func=mybir.ActivationFunctionType.Square,
                        accum_out=dis_sb[:rows, t : t + 1],
                    )

            nc.sync.dma_start(out=dis[:, :], in_=dis_sb[:, :])

    nc.compile()
    _CACHE["nc"] = nc
    return nc


def _run(in_maps, **kwargs):
    from concourse.bass_utils import run_bass_kernel_spmd

    nc = _build()
    return run_bass_kernel_spmd(nc, in_maps, core_ids=list(range(N_CORES)), **kwargs)


def make_in_maps(in_vel, db_vel):
    in_vel = np.ascontiguousarray(in_vel, dtype=np.float32)
    return [
        {
            "db": np.ascontiguousarray(db_vel[c * ROWS : (c + 1) * ROWS]),
            "q": in_vel,
        }
        for c in range(N_CORES)
    ]


def postprocess(results, pred_vel):
    """results: list of 8 dicts with 'dis' [128, 98] -> pred_vel row [1, D_PRED]."""
    # dis_sb[p, t] is the distance of shard row t*128 + p.
    per_core = [
        np.transpose(results[c]["dis"]).reshape(-1)[:ROWS] for c in range(N_CORES)
    ]
    flat = np.concatenate(per_core)
    ind = int(np.argmin(flat))
    return pred_vel[ind][None, :]


def kernel(in_vel, db_vel, pred_vel):
    res = _run(make_in_maps(in_vel, db_vel))
    return postprocess(res.results, pred_vel)


# revision 2
# speedup vs baseline: 1.1923x; 1.1923x over previous
"""Top-1 nearest-neighbor retrieval kernel for Trainium2 (8 NeuronCores).

Reference computation:
    dis = sum((db_vel - in_vel)**2, axis=1)   # [N]
    ind = argmin(dis)
    out = pred_vel[ind][None, :]

Strategy (memory-bound):
  - Shard db_vel row-wise: 100000 rows -> 8 cores x 12500 rows.
  - Each core streams its 12500x1056 f32 shard from HBM in large DMAs and
    computes per-row squared distance to the query with ONE fused custom
    DVE op per 128-row tile:
        out = (db - q_bcast)**2 ; accum_out = row-sum(out)
    (registered at runtime as SQDIFF_ACC_ANT; a single 1x-rate pass over
    the data, so the vector engine runs well under the ~358 GB/s/core HBM
    roofline and the kernel is DMA-bound).
  - Each core writes a tiny [128, 98] distance tile; the host does the final
    argmin over 100k scalars and gathers the pred_vel row (pred_vel never
    needs to touch the device - only one of its rows is ever read).
"""

import numpy as np

N_DB = 100000
D_IN = 1056
N_CORES = 8
ROWS = N_DB // N_CORES          # 12500 rows per core
P = 128                         # SBUF partitions
NTILES = (ROWS + P - 1) // P    # 98 row-tiles per core (97 full + 84 rows)
CHUNK = 7                       # row-tiles per DMA (3.78 MB per transfer)
NCHUNKS = NTILES // CHUNK       # 14

_CACHE = {}


def _get_sqdiff_op():
    """Register (once) a fused custom DVE op:
        out = (in0 - in1)**2 ; accum_out = sum(out, axis=free)
    One 1x-rate vector pass computes the whole squared distance."""
    if "op" in _CACHE:
        return _CACHE["op"]

    from operator import add

    from concourse import dve_ops
    from concourse.dve_spec import Spec, Src0, Src1, Zero, _has_src1, lower, sq
    from concourse.dve_table_gen import dve_ver_for
    from concourse.dve_uop import DveOpSpec

    NAME = "SQDIFF_ACC_ANT"

    def _ref(in0, in1, c0, c1, c2):
        b = ((in0.astype(np.float32) - in1) ** 2).astype(np.float32)
        return b, b.reshape(b.shape[0], -1).sum(axis=-1, keepdims=True)

    spec = Spec(body=sq(Src0 - Src1), accum=add, accum_init=Zero, reference=_ref)

    existing = {o.name: o for o in dve_ops.OPS}
    if NAME in existing:
        op = existing[NAME]
    else:
        row = max(dve_ops._SUB_OPCODE_FOR_NAME.values()) + 1
        dve_ops._SUB_OPCODE_FOR_NAME[NAME] = row
        shas = {}
        for ver in ("v3", "v4"):
            try:
                uops = lower(spec, ver=ver)
                shas[ver] = DveOpSpec(
                    name=NAME, opcode=row, uops=uops, rd1_en=_has_src1(spec)
                ).sha(ver)
            except Exception:
                pass
        op = dve_ops.DveOp(NAME, spec, subdim=False, uops_sha=shas)
        dve_ops.OPS.append(op)
        dve_ops.CUSTOM_DVE_SPECS[NAME] = spec

    _CACHE["op"] = op
    return op


def _build():
    """Trace + compile the per-core Bass program (cached)."""
    if "nc" in _CACHE:
        return _CACHE["nc"]

    import concourse.bacc as bacc
    import concourse.mybir as mybir
    from concourse.tile import TileContext

    sqdiff = _get_sqdiff_op()

    nc = bacc.Bacc(trn_type="TRN2", debug=False)
    db = nc.dram_tensor("db", [ROWS, D_IN], mybir.dt.float32, kind="ExternalInput").ap()
    q = nc.dram_tensor("q", [1, D_IN], mybir.dt.float32, kind="ExternalInput").ap()
    dis = nc.dram_tensor(
        "dis", [P, NTILES], mybir.dt.float32, kind="ExternalOutput"
    ).ap()

    with TileContext(nc) as tc:
        with (
            tc.tile_pool(name="const", bufs=1) as cpool,
            tc.tile_pool(name="chunks", bufs=4) as chpool,
            tc.tile_pool(name="work", bufs=3) as wpool,
            tc.tile_pool(name="acc", bufs=1) as apool,
        ):
            # Broadcast the query across all 128 partitions once.
            qb = cpool.tile([P, D_IN], mybir.dt.float32)
            nc.sync.dma_start(out=qb[:1, :], in_=q[:, :])
            nc.gpsimd.partition_broadcast(qb[:, :], qb[:1, :])

            dis_sb = apool.tile([P, NTILES], mybir.dt.float32)
            nc.vector.memset(dis_sb[:, :], 0.0)

            for c in range(NCHUNKS):
                r0 = c * CHUNK * P
                chunk = chpool.tile([P, CHUNK, D_IN], mybir.dt.float32)
                if c < NCHUNKS - 1:
                    nc.sync.dma_start(
                        out=chunk[:, :, :],
                        in_=db[r0 : r0 + CHUNK * P, :].rearrange(
                            "(a p) d -> p a d", p=P
                        ),
                    )
                    subtiles = [(a, P) for a in range(CHUNK)]
                else:
                    # Tail: 6 full tiles + one 84-row tile (12500 = 97*128+84).
                    full = CHUNK - 1
                    tail = ROWS - (r0 + full * P)
                    nc.sync.dma_start(
                        out=chunk[:, :full, :],
                        in_=db[r0 : r0 + full * P, :].rearrange(
                            "(a p) d -> p a d", p=P
                        ),
                    )
                    nc.sync.dma_start(
                        out=chunk[:tail, full, :],
                        in_=db[r0 + full * P : ROWS, :],
                    )
                    subtiles = [(a, P) for a in range(full)] + [(full, tail)]

                for a, rows in subtiles:
                    t = c * CHUNK + a
                    sq_scr = wpool.tile([P, D_IN], mybir.dt.float32, tag="sq")
                    nc.vector._custom_dve(
                        sqdiff,
                        out=sq_scr[:rows, :],
                        in0=chunk[:rows, a, :],
                        in1=qb[:rows, :],
                        accum_out=dis_sb[:rows, t : t + 1],
                    )

            nc.sync.dma_start(out=dis[:, :], in_=dis_sb[:, :])

    nc.compile()
    _CACHE["nc"] = nc
    return nc


def _run(in_maps, **kwargs):
    from concourse.bass_utils import run_bass_kernel_spmd

    nc = _build()
    return run_bass_kernel_spmd(nc, in_maps, core_ids=list(range(N_CORES)), **kwargs)


def make_in_maps(in_vel, db_vel):
    in_vel = np.ascontiguousarray(in_vel, dtype=np.float32)
    return [
        {
            "db": np.ascontiguousarray(db_vel[c * ROWS : (c + 1) * ROWS]),
            "q": in_vel,
        }
        for c in range(N_CORES)
    ]


def postprocess(results, pred_vel):
    """results: list of 8 dicts with 'dis' [128, 98] -> pred_vel row [1, D_PRED]."""
    # dis_sb[p, t] is the distance of shard row t*128 + p.
    per_core = [
        np.transpose(results[c]["dis"]).reshape(-1)[:ROWS] for c in range(N_CORES)
    ]
    flat = np.concatenate(per_core)
    ind = int(np.argmin(flat))
    return pred_vel[ind][None, :]


def kernel(in_vel, db_vel, pred_vel):
    res = _run(make_in_maps(in_vel, db_vel))
    return postprocess(res.results, pred_vel)
